# revision 22
# baseline (speedup 1.0000x reference)
"""MultiHeadAttention Trainium2 kernel.

B=2, S=2048, E=1024, H=16, D=64. 8 NeuronCores.

Sharding: B*H = 32 (batch, head) pairs -> 4 heads per core (core c handles
batch c//4, heads 4*(c%4)..4*(c%4)+3). Out-projection is column-sharded by
head (Wo folded with Wv on the HOST into wovT); partial [S, E] outputs are
summed on host (the "all-reduce"), which also adds bo once.

Math (per head h):
  S_scores = (q @ Wq.T) @ (k @ Wk.T).T / sqrt(D)  ==  q @ A @ k.T,
    A = Wq.T @ Wk / sqrt(D)  (folded into kaT on host)
  P = softmax(mask(S_scores))  (unnormalized exp + ones-column trick)
  ctx = P @ v  (raw v; Wv folded into wovT)
  out_h = ctxN_h @ wovT_h,  wovT_h = (Wo[:, cols_h] @ Wv).T  (host-computed)

Device layout: scores computed transposed, S.T[sk, sq] tiles, so that
exp(S.T) feeds the ctx matmul directly as the moving operand. va carries a
trailing ones-column, so the softmax denominators r[sq] land on partition
64 of the ctx accumulator (engine ops need 32-aligned partition starts;
the custom DVE reciprocal and gpsimd partition_broadcast additionally
corrupt non-partition-0 operands on HW, hence the copy to partition 0).

Dtypes: all SBUF matmul operands are bf16 (qT, kaT, va, es, ctxN, wovT);
PSUM accumulation is fp32; output is shipped bf16 and summed in fp32 on
host. rel-err budget is 2e-2; measured ~1e-3.

Engine budget per core (steady state): PE ~76us (scores+ctx+outproj),
Act ~74us (exp), DVE ~27us (masks+normalize), Pool ~48us (broadcasts+PSUM
evictions). The emission pipeline runs the PE LOOK units ahead of the ctx
matmuls so the tensor engine never drains; out-projection pieces are
spread between units; PSUM->SBUF evictions ride the Pool engine.
"""

import sys

if "/opt/trn_rl_repo" not in sys.path:
    sys.path.insert(0, "/opt/trn_rl_repo")

import os as _os
from collections import deque
from contextlib import ExitStack

import numpy as np
import ml_dtypes

import concourse.bass as bass
import concourse.tile as tile
from concourse import bacc, mybir
from concourse.bass_utils import run_bass_kernel_spmd

B, S, E, H = 2, 2048, 1024, 16
D = E // H  # 64
N_CORES = 8
HEADS_PER_CORE = H * B // N_CORES  # 4
N_CHUNK = 4  # sq chunks of 512
CHUNK = S // N_CHUNK  # 512
N_BLK = S // 128  # 16 sk blocks of 128
F32 = mybir.dt.float32
BF16 = mybir.dt.bfloat16
NP_BF16 = ml_dtypes.bfloat16


def _analyze_mask(mask):
    """Classify each (sq-chunk, sk-block) region of the shared mask.

    Returns (schedule, tiles): schedule[ci] is a list of (blk, mode, aux)
    with mode in {"plain", "causal", "tile"}; tiles is the list of distinct
    float [128, CHUNK] (sk, sq) multiplicative mask tiles for "tile" mode.
    """
    m = np.asarray(mask).reshape(S, S) != 0
    schedule = []
    tiles = []
    tile_index = {}
    for ci in range(N_CHUNK):
        q0 = ci * CHUNK
        blks = []
        for k in range(N_BLK):
            k0 = k * 128
            mb = m[q0 : q0 + CHUNK, k0 : k0 + 128]  # [sq, sk]
            if not mb.any():
                continue
            if mb.all():
                blks.append((k, "plain", None))
                continue
            causal = (
                np.arange(q0, q0 + CHUNK)[:, None] >= np.arange(k0, k0 + 128)[None, :]
            )
            if np.array_equal(mb, causal):
                blks.append((k, "causal", None))
            else:
                t = np.ascontiguousarray(mb.T.astype(np.float32))  # [sk, sq]
                key = t.tobytes()
                if key not in tile_index:
                    tile_index[key] = len(tiles)
                    tiles.append(t)
                blks.append((k, "tile", tile_index[key]))
        schedule.append(blks)
    return schedule, tiles


def build_nc(schedule, n_mask_tiles, repeat=1, hw_loop=0):
    """Build the SPMD Bass program (identical for all 8 cores).

    repeat>1 / hw_loop>0 re-execute the whole data path (input DMAs
    included) that many times in one NEFF; used by test.py to measure
    per-execution device time as a wall-clock slope.
    """
    nc = bacc.Bacc(
        "TRN2", target_bir_lowering=False, debug=False, num_devices=N_CORES
    )

    qT_d = nc.dram_tensor("qT", [2, 128, S], BF16, kind="ExternalInput").ap()
    kaT_d = nc.dram_tensor("kaT", [2, 128, S], BF16, kind="ExternalInput").ap()
    va_d = nc.dram_tensor(
        "va", [4, 128, N_BLK * (D + 1)], BF16, kind="ExternalInput"
    ).ap()
    wovT_d = nc.dram_tensor("wovT", [2, 128, E], BF16, kind="ExternalInput").ap()
    cm_d = nc.dram_tensor("cmask", [128, 256], BF16, kind="ExternalInput").ap()
    if n_mask_tiles:
        mt_d = nc.dram_tensor(
            "mtiles", [n_mask_tiles, 128, CHUNK], BF16, kind="ExternalInput"
        ).ap()
    out_d = nc.dram_tensor("out", [S, E], BF16, kind="ExternalOutput").ap()
    # DRAM bounce buffer for the 1/r partition-broadcast (DMA cannot read
    # an SBUF AP with partition-stride 0, but a DRAM row re-read can).
    rbc_d = nc.dram_tensor("rbc_scratch", [1, CHUNK], F32, kind="Internal").ap()

    _dbg = bool(int(_os.environ.get("K_DEBUG", "0"))) and not hw_loop and repeat == 1
    if _dbg:
        dbg_es_d = nc.dram_tensor("dbg_es", [128, 1024], BF16, kind="ExternalOutput").ap()
        dbg_r_d = nc.dram_tensor("dbg_r", [1, CHUNK], F32, kind="ExternalOutput").ap()
        dbg_cn_d = nc.dram_tensor("dbg_cn", [128, CHUNK], BF16, kind="ExternalOutput").ap()

    Exp = mybir.ActivationFunctionType.Exp
    MUL = mybir.AluOpType.mult

    LOOK = int(_os.environ.get("K_LOOK", "4"))
    # GPSIMD cannot touch PSUM, and its SW element-ops cost ~3.5us each on
    # HW (6x the cost model) — keep ALL element work off Pool.
    MASK_ENG = _os.environ.get("K_MASK_ENG", "vector")
    # partition-broadcast of 1/r: SBUF->SBUF DMA with a stride-0 input AP
    # (rides the sync HWDGE ring, off all compute engines); "gpsimd" falls
    # back to the Pool software op.
    BCAST = _os.environ.get("K_BCAST", "dma")

    with tile.TileContext(nc) as tc, ExitStack() as ctx:
        const = ctx.enter_context(tc.tile_pool(name="const", bufs=1))
        # bufs=2 double-buffers the input tiles across hw_loop iterations:
        # iteration n+1's input DMAs land while iteration n still computes
        _dbuf = 2 if (hw_loop and int(_os.environ.get("K_DBUF", "0"))) else 1
        qk = ctx.enter_context(tc.tile_pool(name="qk", bufs=_dbuf))
        va_pool = ctx.enter_context(tc.tile_pool(name="vap", bufs=_dbuf))
        es_pool = ctx.enter_context(tc.tile_pool(name="es", bufs=6))
        nrm = ctx.enter_context(tc.tile_pool(name="nrm", bufs=2))
        outp = ctx.enter_context(tc.tile_pool(name="outp", bufs=3))
        # PSUM banks: sp 2x[128,1024] (4) + cxp h0,h1 (2) + mp o x2 (2) = 8
        sp = ctx.enter_context(tc.tile_pool(name="sp", bufs=2, space="PSUM"))
        cxp = ctx.enter_context(tc.tile_pool(name="cxp", bufs=1, space="PSUM"))
        mp = ctx.enter_context(tc.tile_pool(name="mp", bufs=1, space="PSUM"))

        # ---- constants ----
        cmask_sb = const.tile([128, 256], BF16, tag="cmask")
        nc.gpsimd.dma_start(cmask_sb[:], cm_d[:])

        wovT = []
        mtiles = []

        def _emit_prep():
            for p in range(2):
                t = const.tile([128, E], BF16, tag=f"wovT{p}", name=f"wovT{p}")
                nc.gpsimd.dma_start(t[:], wovT_d[p])
                wovT.append(t)
            for i in range(n_mask_tiles):
                t = const.tile([128, CHUNK], BF16, tag=f"mt{i}", name=f"mt{i}")
                nc.gpsimd.dma_start(t[:], mt_d[i])
                mtiles.append(t)

        def _emit_body(_first):
            # ---- input loads, ci-major. All recurring loads ride the SP
            # (sync) HWDGE ring: SWDGE rings occupy the host engine's
            # in-order queue (a [128,512] descriptor-gen costs ~1us of
            # engine time), which stalled the first exp ~8us in an earlier
            # revision. va rides the Pool ring (4 transfers/iter, Pool has
            # slack); chunk-0 operands are emitted first so unit 0 can
            # start ~1.5us in. ----
            qT = []
            kAT = []
            va = []
            for p in range(2):
                qT.append(qk.tile([128, S], BF16, tag=f"qT{p}", name=f"qT{p}"))
                kAT.append(qk.tile([128, S], BF16, tag=f"kAT{p}", name=f"kAT{p}"))
            for h in range(4):
                v_sb = va_pool.tile(
                    [128, N_BLK * (D + 1)], BF16, tag=f"va{h}", name=f"va{h}"
                )
                va.append(v_sb)
            for ci in range(N_CHUNK):
                cs = slice(ci * CHUNK, (ci + 1) * CHUNK)
                for p in range(2):
                    nc.sync.dma_start(kAT[p][:, cs], kaT_d[p, :, cs])
                    nc.sync.dma_start(qT[p][:, cs], qT_d[p, :, cs])
                if ci < 2:
                    for hh in range(2):
                        h = 2 * ci + hh
                        nc.sync.dma_start(va[h][:], va_d[h])

            if _first and not hw_loop:
                _emit_prep()

            # ---- flat unit list ----
            # unit = (ci, p, blk, mode, aux, first_of_cp, last_of_cp)
            units = []
            for ci in range(N_CHUNK):
                blks = schedule[ci]
                for p in range(2):
                    for bi, (blk, mode, aux) in enumerate(blks):
                        units.append(
                            (ci, p, blk, mode, aux, bi == 0, bi == len(blks) - 1)
                        )

            def c0cm(ci, blk, mode):
                # causal block: columns < c0 fully invalid (skip exp/matmul),
                # [c0, c0+128) triangular (post-exp mask), rest fully valid.
                # bf16 matmuls run 1 cyc/row at any width, so cm = c0 exactly.
                if mode != "causal":
                    return 0, 0
                c0 = max(0, blk * 128 - ci * CHUNK)
                return c0, c0

            # per-unit state handed from scores to ctx
            es_of = {}
            ctx_tiles = {}  # (ci, p) -> [h0_tile, h1_tile]
            ctxN_of = {}  # ci -> [ctxN_p0, ctxN_p1]
            pending = deque()
            evict_flip = [0]

            def emit_scores(u):
                ci, p, blk, mode, aux, first, last = u
                q0 = ci * CHUNK
                c0, cm = c0cm(ci, blk, mode)
                s_ps = sp.tile([128, 2 * CHUNK], F32, tag="s", name="s_ps")
                es = es_pool.tile([128, 2 * CHUNK], BF16, tag="es", name="es")
                es_of[id(u)] = (s_ps, es)
                for hh in range(2):
                    o = hh * D
                    nc.tensor.matmul(
                        s_ps[:, hh * CHUNK + cm : (hh + 1) * CHUNK],
                        kAT[p][o : o + D, blk * 128 : (blk + 1) * 128],
                        qT[p][o : o + D, q0 + cm : q0 + CHUNK],
                        start=True,
                        stop=True,
                    )
                # single exp instruction covering both heads
                if mode == "causal" and cm > 0:
                    es3 = es[:].rearrange("p (h w) -> p h w", h=2)
                    sp3 = s_ps[:].rearrange("p (h w) -> p h w", h=2)
                    nc.scalar.activation(
                        es3[:, :, cm:CHUNK], sp3[:, :, cm:CHUNK], Exp
                    )
                else:
                    nc.scalar.activation(es[:], s_ps[:], Exp)

            def emit_ctx(u):
                ci, p, blk, mode, aux, first, last = u
                c0, cm = c0cm(ci, blk, mode)
                s_ps, es = es_of.pop(id(u))
                _me = getattr(nc, MASK_ENG)
                if mode == "causal":
                    # zero the triangular region (post-exp) for both heads
                    es3 = es[:].rearrange("p (h w) -> p h w", h=2)
                    _me.tensor_tensor(
                        es3[:, :, c0 : c0 + 128],
                        es3[:, :, c0 : c0 + 128],
                        cmask_sb[:, 0:256],
                        op=MUL,
                    )
                elif mode == "tile":
                    for hh in range(2):
                        _me.tensor_tensor(
                            es[:, hh * CHUNK : (hh + 1) * CHUNK],
                            es[:, hh * CHUNK : (hh + 1) * CHUNK],
                            mtiles[aux][:],
                            op=MUL,
                        )
                if first:
                    ctx_tiles[(ci, p)] = [
                        cxp.tile([D + 1, CHUNK], F32, tag=f"h{hh}", name=f"ctx{hh}")
                        for hh in range(2)
                    ]
                ctx_ps = ctx_tiles[(ci, p)]
                for hh in range(2):
                    h = 2 * p + hh
                    nc.tensor.matmul(
                        ctx_ps[hh][:, cm:],
                        va[h][:, blk * (D + 1) : (blk + 1) * (D + 1)],
                        es[:, hh * CHUNK + cm : (hh + 1) * CHUNK],
                        start=first,
                        stop=last,
                    )
                if _dbg and ci == 0 and p == 0 and blk == int(_os.environ.get("K_DBG_BLK", "0")):
                    nc.sync.dma_start(dbg_es_d[:], es[:])
                if last:
                    emit_normalize(ci, p)

            def emit_normalize(ci, p):
                # ctx_ps rows: 0..63 unnormalized ctx, row 64 = softmax
                # denominator r (trailing ones-column in va). r is copied to
                # partition 0 (Pool engine; DVE does recip+mul) because the
                # custom DVE reciprocal and gpsimd partition_broadcast
                # corrupt non-partition-0 operands on HW.
                ctx_ps = ctx_tiles.pop((ci, p))
                ctxN_p = nrm.tile(
                    [128, CHUNK], BF16, tag=f"ctxN{p}", name=f"ctxN{p}"
                )
                ctxN_of.setdefault(ci, [None, None])[p] = ctxN_p
                for hh in range(2):
                    o = hh * D
                    rr = nrm.tile([1, CHUNK], F32, tag="rr")
                    nc.vector.tensor_copy(rr[:], ctx_ps[hh][D : D + 1, :])
                    r_inv = nrm.tile([1, CHUNK], F32, tag="rinv")
                    nc.vector.reciprocal_approx_fast(out=r_inv[:], in_=rr[:])
                    r_bc = nrm.tile([D, CHUNK], F32, tag="rbc")
                    if BCAST in ("dma", "dmag"):
                        # bounce through DRAM: the replicated re-read does
                        # the partition broadcast inside the DMA engine.
                        # Both transfers share one ring, so write->read
                        # ordering is the queue order.
                        _br = nc.sync if BCAST == "dma" else nc.gpsimd
                        _br.dma_start(rbc_d[:], r_inv[:])
                        _br.dma_start(r_bc[:], rbc_d[:].to_broadcast((D, CHUNK)))
                    else:
                        nc.gpsimd.partition_broadcast(r_bc[:], r_inv[:])
                    nc.vector.tensor_tensor(
                        ctxN_p[o : o + D, :],
                        ctx_ps[hh][0:D, :],
                        r_bc[:],
                        op=MUL,
                    )
                    if _dbg and ci == 0 and p == 0 and hh == 0:
                        nc.sync.dma_start(dbg_r_d[:], r_inv[:])
                if _dbg and ci == 0 and p == 0:
                    nc.sync.dma_start(dbg_cn_d[:], ctxN_p[:])
                if p == 1:
                    for pc in outp_pieces(ci):
                        pending.append(pc)

            def outp_pieces(ci):
                q0 = ci * CHUNK
                for sb in range(CHUNK // 128):
                    for ec in range(E // 512):

                        def piece(sb=sb, ec=ec, q0=q0, ci=ci):
                            ctxN = ctxN_of[ci]
                            ls = slice(sb * 128, (sb + 1) * 128)
                            es_ = slice(ec * 512, (ec + 1) * 512)
                            o_ps = mp.tile(
                                [128, 512], F32, tag="o", name="o_ps", bufs=2
                            )
                            nc.tensor.matmul(
                                o_ps[:],
                                ctxN[0][:, ls],
                                wovT[0][:, es_],
                                start=True,
                                stop=False,
                            )
                            nc.tensor.matmul(
                                o_ps[:],
                                ctxN[1][:, ls],
                                wovT[1][:, es_],
                                start=False,
                                stop=True,
                            )
                            o_sb = outp.tile([128, 512], BF16, tag="osb", name="o_sb")
                            # steady state: DVE evicts (Act is exp-saturated);
                            # tail (last chunk): Act is idle, split with it
                            if ci == N_CHUNK - 1 and evict_flip[0] % 2 == 0:
                                nc.scalar.copy(o_sb[:], o_ps[:])
                            else:
                                nc.vector.tensor_copy(o_sb[:], o_ps[:])
                            evict_flip[0] += 1
                            _or = getattr(nc, _os.environ.get("K_OUT_RING", "sync"))
                            _or.dma_start(
                                out_d[q0 + sb * 128 : q0 + (sb + 1) * 128, es_],
                                o_sb[:],
                            )

                        yield piece

            # ---- pipelined emission: PE runs LOOK units ahead of ctx so
            # by the time PE reaches ctx(u) the exp/mask of u finished long
            # ago and the tensor engine never drains ----
            for i, u in enumerate(units):
                emit_scores(u)
                for _ in range(2):
                    if pending:
                        pending.popleft()()
                if i >= LOOK:
                    emit_ctx(units[i - LOOK])
            for u in units[-LOOK:]:
                emit_ctx(u)
            while pending:
                pending.popleft()()

        if hw_loop:
            _emit_prep()
            with tc.For_i(0, hw_loop) as _i:
                _emit_body(False)
        else:
            for _rep in range(repeat):
                _emit_body(_rep == 0)

    nc.compile()
    return nc


def _canonical_cmask():
    # [sk, sq] triangle, tiled twice (one copy per head): valid iff sq >= sk
    i = np.arange(128)[:, None]
    m128 = (np.arange(128)[None, :] >= i).astype(np.float32)
    return np.tile(m128, (1, 2))  # [128, 256]


def prepare(key, query, value, mask, Wq, Wk, Wv, Wo, bo, build=True):
    """Host-side sharding/layout prep. Returns (nc, in_maps, gather)."""
    key = np.asarray(key, dtype=np.float32)
    query = np.asarray(query, dtype=np.float32)
    value = np.asarray(value, dtype=np.float32)
    Wq = np.asarray(Wq, dtype=np.float32)
    Wk = np.asarray(Wk, dtype=np.float32)
    Wv = np.asarray(Wv, dtype=np.float32)
    Wo = np.asarray(Wo, dtype=np.float32)
    bo = np.asarray(bo, dtype=np.float32)

    schedule, mtiles = _analyze_mask(mask)
    nc = build_nc(schedule, len(mtiles)) if build else None

    A = (Wq.T @ Wk) / np.float32(np.sqrt(D))  # scores = q @ A @ k.T
    cmask = _canonical_cmask().astype(NP_BF16)
    mt = np.stack(mtiles).astype(NP_BF16) if mtiles else None
    # wovT[p][hh*D + d, e] = sum_d' Wv[d', d] * Wo[e, (2p+hh)*D + d']
    wovT_all = np.stack(
        [
            np.concatenate(
                [
                    (Wo[:, (2 * p + hh) * D : (2 * p + hh + 1) * D] @ Wv).T
                    for hh in range(2)
                ],
                axis=0,
            )
            for p in range(H // 2)
        ]
    )  # [H//2, 128, E]

    in_maps = []
    for c in range(N_CORES):
        b = c // 4
        h0 = 4 * (c % 4)
        hs = slice(h0, h0 + 4)
        q = query[b].reshape(S, H, D)[:, hs, :]  # [S, 4, D]
        k = key[b].reshape(S, H, D)[:, hs, :]
        v = value[b].reshape(S, H, D)[:, hs, :]
        # pair-stacked transposed layouts [2, 128, S]; A folded into k
        qT = np.ascontiguousarray(q.transpose(1, 2, 0).reshape(2, 2 * D, S))
        kaT = np.ascontiguousarray(
            np.einsum("de,she->hds", A, k, dtype=np.float32, casting="same_kind")
            .reshape(2, 2 * D, S)
            .astype(np.float32)
        )
        # trailing ones-column: r lands on (32-aligned) partition 64
        va = np.ones((4, S, D + 1), dtype=np.float32)
        va[:, :, :D] = v.transpose(1, 0, 2)
        # partition-major: [4, S, D+1] -> [4, 128, N_BLK*(D+1)]
        va = va.reshape(4, N_BLK, 128, D + 1).transpose(0, 2, 1, 3).reshape(
            4, 128, N_BLK * (D + 1)
        )
        m = {
            "qT": qT.astype(NP_BF16),
            "kaT": kaT.astype(NP_BF16),
            "va": np.ascontiguousarray(va).astype(NP_BF16),
            "wovT": wovT_all[2 * (c % 4) : 2 * (c % 4) + 2].astype(NP_BF16),
            "cmask": cmask,
        }
        if mt is not None:
            m["mtiles"] = mt
        in_maps.append(m)

    def gather(results):
        out = np.empty((B, S, E), dtype=np.float32)
        for b in range(B):
            acc = results[4 * b]["out"].astype(np.float32)
            for c in range(4 * b + 1, 4 * b + 4):
                acc = acc + results[c]["out"].astype(np.float32)
            out[b] = acc + bo[None, :]
        return out

    return nc, in_maps, gather


def kernel(key, query, value, mask, Wq, Wk, Wv, Wo, bo):
    nc, in_maps, gather = prepare(key, query, value, mask, Wq, Wk, Wv, Wo, bo)
    res = run_bass_kernel_spmd(nc, in_maps, core_ids=list(range(N_CORES)))
    return gather(res.results)


# revision 24
# speedup vs baseline: 1.4710x; 1.4710x over previous
"""MultiHeadAttention Trainium2 kernel.

B=2, S=2048, E=1024, H=16, D=64. 8 NeuronCores.

Sharding: B*H = 32 (batch, head) pairs -> 4 heads per core (core c handles
batch c//4, heads 4*(c%4)..4*(c%4)+3). Out-projection is column-sharded by
head (Wo folded with Wv on the HOST into wovT); partial [S, E] outputs are
summed on host (the "all-reduce"), which also adds bo once.

Math (per head h):
  S_scores = (q @ Wq.T) @ (k @ Wk.T).T / sqrt(D)  ==  q @ A @ k.T,
    A = Wq.T @ Wk / sqrt(D)  (folded into kaT on host)
  P = softmax(mask(S_scores))  (unnormalized exp + ones-column trick)
  ctx = P @ v  (raw v; Wv folded into wovT)
  out_h = ctxN_h @ wovT_h,  wovT_h = (Wo[:, cols_h] @ Wv).T  (host-computed)

Device layout: scores computed transposed, S.T[sk, sq] tiles, so that
exp(S.T) feeds the ctx matmul directly as the moving operand. va carries a
trailing ones-column, so the softmax denominators r[sq] land on partition
64 of the ctx accumulator (engine ops need 32-aligned partition starts;
the custom DVE reciprocal and gpsimd partition_broadcast additionally
corrupt non-partition-0 operands on HW, hence the copy to partition 0).

Dtypes: all SBUF matmul operands are bf16 (qT, kaT, va, es, ctxN, wovT);
PSUM accumulation is fp32; output is shipped bf16 and summed in fp32 on
host. rel-err budget is 2e-2; measured ~1e-3.

Engine budget per core (steady state): PE ~76us (scores+ctx+outproj),
Act ~74us (exp), DVE ~27us (masks+normalize), Pool ~48us (broadcasts+PSUM
evictions). The emission pipeline runs the PE LOOK units ahead of the ctx
matmuls so the tensor engine never drains; out-projection pieces are
spread between units; PSUM->SBUF evictions ride the Pool engine.
"""

import sys

if "/opt/trn_rl_repo" not in sys.path:
    sys.path.insert(0, "/opt/trn_rl_repo")

import os as _os
from collections import deque
from contextlib import ExitStack

import numpy as np
import ml_dtypes

import concourse.bass as bass
import concourse.tile as tile
from concourse import bacc, mybir
from concourse.bass_utils import run_bass_kernel_spmd

B, S, E, H = 2, 2048, 1024, 16
D = E // H  # 64
N_CORES = 8
HEADS_PER_CORE = H * B // N_CORES  # 4
N_CHUNK = 4  # sq chunks of 512
CHUNK = S // N_CHUNK  # 512
N_BLK = S // 128  # 16 sk blocks of 128
F32 = mybir.dt.float32
BF16 = mybir.dt.bfloat16
NP_BF16 = ml_dtypes.bfloat16


def _analyze_mask(mask):
    """Classify each (sq-chunk, sk-block) region of the shared mask.

    Returns (schedule, tiles): schedule[ci] is a list of (blk, mode, aux)
    with mode in {"plain", "causal", "tile"}; tiles is the list of distinct
    float [128, CHUNK] (sk, sq) multiplicative mask tiles for "tile" mode.
    """
    m = np.asarray(mask).reshape(S, S) != 0
    schedule = []
    tiles = []
    tile_index = {}
    for ci in range(N_CHUNK):
        q0 = ci * CHUNK
        blks = []
        for k in range(N_BLK):
            k0 = k * 128
            mb = m[q0 : q0 + CHUNK, k0 : k0 + 128]  # [sq, sk]
            if not mb.any():
                continue
            if mb.all():
                blks.append((k, "plain", None))
                continue
            causal = (
                np.arange(q0, q0 + CHUNK)[:, None] >= np.arange(k0, k0 + 128)[None, :]
            )
            if np.array_equal(mb, causal):
                blks.append((k, "causal", None))
            else:
                t = np.ascontiguousarray(mb.T.astype(np.float32))  # [sk, sq]
                key = t.tobytes()
                if key not in tile_index:
                    tile_index[key] = len(tiles)
                    tiles.append(t)
                blks.append((k, "tile", tile_index[key]))
        schedule.append(blks)
    return schedule, tiles


def build_nc(schedule, n_mask_tiles, repeat=1, hw_loop=0):
    """Build the SPMD Bass program (identical for all 8 cores).

    repeat>1 / hw_loop>0 re-execute the whole data path (input DMAs
    included) that many times in one NEFF; used by test.py to measure
    per-execution device time as a wall-clock slope.
    """
    nc = bacc.Bacc(
        "TRN2", target_bir_lowering=False, debug=False, num_devices=N_CORES
    )

    qT_d = nc.dram_tensor("qT", [2, 128, S], BF16, kind="ExternalInput").ap()
    kaT_d = nc.dram_tensor("kaT", [2, 128, S], BF16, kind="ExternalInput").ap()
    va_d = nc.dram_tensor(
        "va", [4, 128, N_BLK * (D + 1)], BF16, kind="ExternalInput"
    ).ap()
    wovT_d = nc.dram_tensor("wovT", [2, 128, E], BF16, kind="ExternalInput").ap()
    cm_d = nc.dram_tensor("cmask", [128, 256], BF16, kind="ExternalInput").ap()
    if n_mask_tiles:
        mt_d = nc.dram_tensor(
            "mtiles", [n_mask_tiles, 128, CHUNK], BF16, kind="ExternalInput"
        ).ap()
    out_d = nc.dram_tensor("out", [S, E], BF16, kind="ExternalOutput").ap()
    # DRAM bounce buffer for the 1/r partition-broadcast (DMA cannot read
    # an SBUF AP with partition-stride 0, but a DRAM row re-read can).
    rbc_d = nc.dram_tensor("rbc_scratch", [1, CHUNK], F32, kind="Internal").ap()

    _dbg = bool(int(_os.environ.get("K_DEBUG", "0"))) and not hw_loop and repeat == 1
    if _dbg:
        dbg_es_d = nc.dram_tensor("dbg_es", [128, 1024], BF16, kind="ExternalOutput").ap()
        dbg_r_d = nc.dram_tensor("dbg_r", [1, CHUNK], F32, kind="ExternalOutput").ap()
        dbg_cn_d = nc.dram_tensor("dbg_cn", [128, CHUNK], BF16, kind="ExternalOutput").ap()

    Exp = mybir.ActivationFunctionType.Exp
    MUL = mybir.AluOpType.mult

    LOOK = int(_os.environ.get("K_LOOK", "4"))
    # GPSIMD cannot touch PSUM, and its SW element-ops cost ~3.5us each on
    # HW (6x the cost model) — keep ALL element work off Pool.
    MASK_ENG = _os.environ.get("K_MASK_ENG", "vector")
    # partition-broadcast of 1/r: the Pool software op measures fastest on
    # HW; DRAM-bounce DMA variants ("dma"/"dmag") add ~40-60us of chain
    # latency (ring queueing) and lose.
    BCAST = _os.environ.get("K_BCAST", "gpsimd")

    with tile.TileContext(nc) as tc, ExitStack() as ctx:
        const = ctx.enter_context(tc.tile_pool(name="const", bufs=1))
        # bufs=2 double-buffers the input tiles across hw_loop iterations:
        # iteration n+1's input DMAs land while iteration n still computes
        _dbuf = 2 if (hw_loop and int(_os.environ.get("K_DBUF", "0"))) else 1
        qk = ctx.enter_context(tc.tile_pool(name="qk", bufs=_dbuf))
        va_pool = ctx.enter_context(tc.tile_pool(name="vap", bufs=_dbuf))
        es_pool = ctx.enter_context(
            tc.tile_pool(name="es", bufs=int(_os.environ.get("K_ESBUFS", "8")))
        )
        nrm = ctx.enter_context(tc.tile_pool(name="nrm", bufs=2))
        outp = ctx.enter_context(tc.tile_pool(name="outp", bufs=3))
        # PSUM banks: sp 2x[128,1024] (4) + cxp h0,h1 (2) + mp o x2 (2) = 8
        sp = ctx.enter_context(tc.tile_pool(name="sp", bufs=2, space="PSUM"))
        cxp = ctx.enter_context(tc.tile_pool(name="cxp", bufs=1, space="PSUM"))
        mp = ctx.enter_context(tc.tile_pool(name="mp", bufs=1, space="PSUM"))

        # ---- constants ----
        cmask_sb = const.tile([128, 256], BF16, tag="cmask")
        nc.gpsimd.dma_start(cmask_sb[:], cm_d[:])

        wovT = []
        mtiles = []

        def _emit_prep():
            for p in range(2):
                t = const.tile([128, E], BF16, tag=f"wovT{p}", name=f"wovT{p}")
                nc.gpsimd.dma_start(t[:], wovT_d[p])
                wovT.append(t)
            for i in range(n_mask_tiles):
                t = const.tile([128, CHUNK], BF16, tag=f"mt{i}", name=f"mt{i}")
                nc.gpsimd.dma_start(t[:], mt_d[i])
                mtiles.append(t)

        def _emit_body(_first):
            # ---- input loads, ci-major. All recurring loads ride the SP
            # (sync) HWDGE ring: SWDGE rings occupy the host engine's
            # in-order queue (a [128,512] descriptor-gen costs ~1us of
            # engine time), which stalled the first exp ~8us in an earlier
            # revision. va rides the Pool ring (4 transfers/iter, Pool has
            # slack); chunk-0 operands are emitted first so unit 0 can
            # start ~1.5us in. ----
            qT = []
            kAT = []
            va = []
            for p in range(2):
                qT.append(qk.tile([128, S], BF16, tag=f"qT{p}", name=f"qT{p}"))
                kAT.append(qk.tile([128, S], BF16, tag=f"kAT{p}", name=f"kAT{p}"))
            for h in range(4):
                v_sb = va_pool.tile(
                    [128, N_BLK * (D + 1)], BF16, tag=f"va{h}", name=f"va{h}"
                )
                va.append(v_sb)
            for ci in range(N_CHUNK):
                cs = slice(ci * CHUNK, (ci + 1) * CHUNK)
                for p in range(2):
                    nc.sync.dma_start(kAT[p][:, cs], kaT_d[p, :, cs])
                    nc.sync.dma_start(qT[p][:, cs], qT_d[p, :, cs])
                if ci < 2:
                    for hh in range(2):
                        h = 2 * ci + hh
                        nc.sync.dma_start(va[h][:], va_d[h])

            if _first and not hw_loop:
                _emit_prep()

            # ---- flat unit list ----
            # unit = (ci, p, blk, mode, aux, first_of_cp, last_of_cp)
            units = []
            for ci in range(N_CHUNK):
                blks = schedule[ci]
                for p in range(2):
                    for bi, (blk, mode, aux) in enumerate(blks):
                        units.append(
                            (ci, p, blk, mode, aux, bi == 0, bi == len(blks) - 1)
                        )

            def c0cm(ci, blk, mode):
                # causal block: columns < c0 fully invalid (skip exp/matmul),
                # [c0, c0+128) triangular (post-exp mask), rest fully valid.
                # bf16 matmuls run 1 cyc/row at any width, so cm = c0 exactly.
                if mode != "causal":
                    return 0, 0
                c0 = max(0, blk * 128 - ci * CHUNK)
                return c0, c0

            # per-unit state handed from scores to ctx
            es_of = {}
            ctx_tiles = {}  # (ci, p) -> [h0_tile, h1_tile]
            ctxN_of = {}  # ci -> [ctxN_p0, ctxN_p1]
            pending = deque()
            evict_flip = [0]

            def emit_scores(u):
                ci, p, blk, mode, aux, first, last = u
                q0 = ci * CHUNK
                c0, cm = c0cm(ci, blk, mode)
                s_ps = sp.tile([128, 2 * CHUNK], F32, tag="s", name="s_ps")
                es = es_pool.tile([128, 2 * CHUNK], BF16, tag="es", name="es")
                es_of[id(u)] = (s_ps, es)
                for hh in range(2):
                    o = hh * D
                    nc.tensor.matmul(
                        s_ps[:, hh * CHUNK + cm : (hh + 1) * CHUNK],
                        kAT[p][o : o + D, blk * 128 : (blk + 1) * 128],
                        qT[p][o : o + D, q0 + cm : q0 + CHUNK],
                        start=True,
                        stop=True,
                    )
                # single exp instruction covering both heads
                if mode == "causal" and cm > 0:
                    es3 = es[:].rearrange("p (h w) -> p h w", h=2)
                    sp3 = s_ps[:].rearrange("p (h w) -> p h w", h=2)
                    nc.scalar.activation(
                        es3[:, :, cm:CHUNK], sp3[:, :, cm:CHUNK], Exp
                    )
                else:
                    nc.scalar.activation(es[:], s_ps[:], Exp)

            def emit_ctx(u):
                ci, p, blk, mode, aux, first, last = u
                c0, cm = c0cm(ci, blk, mode)
                s_ps, es = es_of.pop(id(u))
                _me = getattr(nc, MASK_ENG)
                if mode == "causal":
                    # zero the triangular region (post-exp) for both heads
                    es3 = es[:].rearrange("p (h w) -> p h w", h=2)
                    _me.tensor_tensor(
                        es3[:, :, c0 : c0 + 128],
                        es3[:, :, c0 : c0 + 128],
                        cmask_sb[:, 0:256],
                        op=MUL,
                    )
                elif mode == "tile":
                    for hh in range(2):
                        _me.tensor_tensor(
                            es[:, hh * CHUNK : (hh + 1) * CHUNK],
                            es[:, hh * CHUNK : (hh + 1) * CHUNK],
                            mtiles[aux][:],
                            op=MUL,
                        )
                if first:
                    ctx_tiles[(ci, p)] = [
                        cxp.tile([D + 1, CHUNK], F32, tag=f"h{hh}", name=f"ctx{hh}")
                        for hh in range(2)
                    ]
                ctx_ps = ctx_tiles[(ci, p)]
                for hh in range(2):
                    h = 2 * p + hh
                    nc.tensor.matmul(
                        ctx_ps[hh][:, cm:],
                        va[h][:, blk * (D + 1) : (blk + 1) * (D + 1)],
                        es[:, hh * CHUNK + cm : (hh + 1) * CHUNK],
                        start=first,
                        stop=last,
                    )
                if _dbg and ci == 0 and p == 0 and blk == int(_os.environ.get("K_DBG_BLK", "0")):
                    nc.sync.dma_start(dbg_es_d[:], es[:])
                if last:
                    emit_normalize(ci, p)

            def emit_normalize(ci, p):
                # ctx_ps rows: 0..63 unnormalized ctx, row 64 = softmax
                # denominator r (trailing ones-column in va). r is copied to
                # partition 0 (Pool engine; DVE does recip+mul) because the
                # custom DVE reciprocal and gpsimd partition_broadcast
                # corrupt non-partition-0 operands on HW.
                ctx_ps = ctx_tiles.pop((ci, p))
                ctxN_p = nrm.tile(
                    [128, CHUNK], BF16, tag=f"ctxN{p}", name=f"ctxN{p}"
                )
                ctxN_of.setdefault(ci, [None, None])[p] = ctxN_p
                for hh in range(2):
                    o = hh * D
                    rr = nrm.tile([1, CHUNK], F32, tag="rr")
                    nc.vector.tensor_copy(rr[:], ctx_ps[hh][D : D + 1, :])
                    r_inv = nrm.tile([1, CHUNK], F32, tag="rinv")
                    nc.vector.reciprocal_approx_fast(out=r_inv[:], in_=rr[:])
                    r_bc = nrm.tile([D, CHUNK], F32, tag="rbc")
                    if BCAST in ("dma", "dmag"):
                        # bounce through DRAM: the replicated re-read does
                        # the partition broadcast inside the DMA engine.
                        # Both transfers share one ring, so write->read
                        # ordering is the queue order.
                        _br = nc.sync if BCAST == "dma" else nc.gpsimd
                        _br.dma_start(rbc_d[:], r_inv[:])
                        _br.dma_start(r_bc[:], rbc_d[:].to_broadcast((D, CHUNK)))
                    else:
                        nc.gpsimd.partition_broadcast(r_bc[:], r_inv[:])
                    nc.vector.tensor_tensor(
                        ctxN_p[o : o + D, :],
                        ctx_ps[hh][0:D, :],
                        r_bc[:],
                        op=MUL,
                    )
                    if _dbg and ci == 0 and p == 0 and hh == 0:
                        nc.sync.dma_start(dbg_r_d[:], r_inv[:])
                if _dbg and ci == 0 and p == 0:
                    nc.sync.dma_start(dbg_cn_d[:], ctxN_p[:])
                if p == 1:
                    for pc in outp_pieces(ci):
                        pending.append(pc)

            def outp_pieces(ci):
                q0 = ci * CHUNK
                for sb in range(CHUNK // 128):
                    for ec in range(E // 512):

                        def piece(sb=sb, ec=ec, q0=q0, ci=ci):
                            ctxN = ctxN_of[ci]
                            ls = slice(sb * 128, (sb + 1) * 128)
                            es_ = slice(ec * 512, (ec + 1) * 512)
                            o_ps = mp.tile(
                                [128, 512], F32, tag="o", name="o_ps", bufs=2
                            )
                            nc.tensor.matmul(
                                o_ps[:],
                                ctxN[0][:, ls],
                                wovT[0][:, es_],
                                start=True,
                                stop=False,
                            )
                            nc.tensor.matmul(
                                o_ps[:],
                                ctxN[1][:, ls],
                                wovT[1][:, es_],
                                start=False,
                                stop=True,
                            )
                            o_sb = outp.tile([128, 512], BF16, tag="osb", name="o_sb")
                            # steady state: DVE evicts (Act is exp-saturated);
                            # tail (last chunk): Act is idle, split with it
                            if ci == N_CHUNK - 1 and evict_flip[0] % 2 == 0:
                                nc.scalar.copy(o_sb[:], o_ps[:])
                            else:
                                nc.vector.tensor_copy(o_sb[:], o_ps[:])
                            evict_flip[0] += 1
                            _or = getattr(nc, _os.environ.get("K_OUT_RING", "sync"))
                            _or.dma_start(
                                out_d[q0 + sb * 128 : q0 + (sb + 1) * 128, es_],
                                o_sb[:],
                            )

                        yield piece

            # ---- pipelined emission: PE runs LOOK units ahead of ctx so
            # by the time PE reaches ctx(u) the exp/mask of u finished long
            # ago and the tensor engine never drains ----
            for i, u in enumerate(units):
                emit_scores(u)
                for _ in range(2):
                    if pending:
                        pending.popleft()()
                if i >= LOOK:
                    emit_ctx(units[i - LOOK])
            for u in units[-LOOK:]:
                emit_ctx(u)
            while pending:
                pending.popleft()()

        if hw_loop:
            _emit_prep()
            with tc.For_i(0, hw_loop) as _i:
                _emit_body(False)
        else:
            for _rep in range(repeat):
                _emit_body(_rep == 0)

    nc.compile()
    return nc


def _canonical_cmask():
    # [sk, sq] triangle, tiled twice (one copy per head): valid iff sq >= sk
    i = np.arange(128)[:, None]
    m128 = (np.arange(128)[None, :] >= i).astype(np.float32)
    return np.tile(m128, (1, 2))  # [128, 256]


def prepare(key, query, value, mask, Wq, Wk, Wv, Wo, bo, build=True):
    """Host-side sharding/layout prep. Returns (nc, in_maps, gather)."""
    key = np.asarray(key, dtype=np.float32)
    query = np.asarray(query, dtype=np.float32)
    value = np.asarray(value, dtype=np.float32)
    Wq = np.asarray(Wq, dtype=np.float32)
    Wk = np.asarray(Wk, dtype=np.float32)
    Wv = np.asarray(Wv, dtype=np.float32)
    Wo = np.asarray(Wo, dtype=np.float32)
    bo = np.asarray(bo, dtype=np.float32)

    schedule, mtiles = _analyze_mask(mask)
    nc = build_nc(schedule, len(mtiles)) if build else None

    A = (Wq.T @ Wk) / np.float32(np.sqrt(D))  # scores = q @ A @ k.T
    cmask = _canonical_cmask().astype(NP_BF16)
    mt = np.stack(mtiles).astype(NP_BF16) if mtiles else None
    # wovT[p][hh*D + d, e] = sum_d' Wv[d', d] * Wo[e, (2p+hh)*D + d']
    wovT_all = np.stack(
        [
            np.concatenate(
                [
                    (Wo[:, (2 * p + hh) * D : (2 * p + hh + 1) * D] @ Wv).T
                    for hh in range(2)
                ],
                axis=0,
            )
            for p in range(H // 2)
        ]
    )  # [H//2, 128, E]

    in_maps = []
    for c in range(N_CORES):
        b = c // 4
        h0 = 4 * (c % 4)
        hs = slice(h0, h0 + 4)
        q = query[b].reshape(S, H, D)[:, hs, :]  # [S, 4, D]
        k = key[b].reshape(S, H, D)[:, hs, :]
        v = value[b].reshape(S, H, D)[:, hs, :]
        # pair-stacked transposed layouts [2, 128, S]; A folded into k
        qT = np.ascontiguousarray(q.transpose(1, 2, 0).reshape(2, 2 * D, S))
        kaT = np.ascontiguousarray(
            np.einsum("de,she->hds", A, k, dtype=np.float32, casting="same_kind")
            .reshape(2, 2 * D, S)
            .astype(np.float32)
        )
        # trailing ones-column: r lands on (32-aligned) partition 64
        va = np.ones((4, S, D + 1), dtype=np.float32)
        va[:, :, :D] = v.transpose(1, 0, 2)
        # partition-major: [4, S, D+1] -> [4, 128, N_BLK*(D+1)]
        va = va.reshape(4, N_BLK, 128, D + 1).transpose(0, 2, 1, 3).reshape(
            4, 128, N_BLK * (D + 1)
        )
        m = {
            "qT": qT.astype(NP_BF16),
            "kaT": kaT.astype(NP_BF16),
            "va": np.ascontiguousarray(va).astype(NP_BF16),
            "wovT": wovT_all[2 * (c % 4) : 2 * (c % 4) + 2].astype(NP_BF16),
            "cmask": cmask,
        }
        if mt is not None:
            m["mtiles"] = mt
        in_maps.append(m)

    def gather(results):
        out = np.empty((B, S, E), dtype=np.float32)
        for b in range(B):
            acc = results[4 * b]["out"].astype(np.float32)
            for c in range(4 * b + 1, 4 * b + 4):
                acc = acc + results[c]["out"].astype(np.float32)
            out[b] = acc + bo[None, :]
        return out

    return nc, in_maps, gather


def kernel(key, query, value, mask, Wq, Wk, Wv, Wo, bo):
    nc, in_maps, gather = prepare(key, query, value, mask, Wq, Wk, Wv, Wo, bo)
    res = run_bass_kernel_spmd(nc, in_maps, core_ids=list(range(N_CORES)))
    return gather(res.results)
